# revision 61
# baseline (speedup 1.0000x reference)
"""Bass/Trainium2 kernel for nn_F_Loss_65446711656630.

Strategy (data-parallel over N, 8 cores, fp8 e4m3 inputs):
  - Host: GLOBAL stable sort of all rows by class id, slice 8192 rows/core.
  - Per core the rows are split between three engine pipelines so all three
    finish together:
      * rows 0:4608 ("A part"), transposed to [128 features x rows] pieces:
        DVE bn_stats in 512-row groups. One DVE pass yields count/mean/M2
        -> BOTH the h-sum and the h^2-sum per granule.
      * rows 4608:8192 ("B part"), natural [row, feature] layout as 28
        chunks of [128 rows x 512 features]: ACT squares each chunk
        (fp8 -> fp16), and the PE contracts the raw chunks (fp8 DoubleRow,
        two chunks per matmul) and the squares (fp16) against per-chunk
        one-hot class matrices, accumulating exact per-class sums in PSUM.
        No class-boundary fixups needed for this half.
  - Host: per-class stats from pure-class bn granules (f64) + exact host
    recompute for granules spanning a class boundary + the PSUM partials;
    then the tiny O(C^2 D) pairwise betainc/top-k stage in f32 jax on CPU
    (mirroring the reference's numerics exactly).

  fp8 e4m3 input costs ~6e-4 final rel err (gate is 2e-2) and halves HBM
  traffic to ~4 MiB/core.  Squares stay fp16: fp8 squares would cost
  ~6e-3.
"""

import numpy as np

C = 16
D = 512
N = 65536
NCORES = 8
ROWS = N // NCORES          # 8192 rows per core
P = 128                     # SBUF partitions
PIECE = 2048                # rows per full layout-A piece
G = 512                     # bn_stats group size (HW max)
GPP = PIECE // G            # 4 groups per piece
NBLK = D // P               # 4 feature blocks
A_ROWS = 4096               # rows handled by bn_stats (layout A)
NPIECE_A = 2                # 2048-row pieces per block
NP_A = NBLK * NPIECE_A      # 8 layout-A pieces
B_ROWS = ROWS - A_ROWS      # 4096 rows handled by PE (layout B)
NCHUNK = B_ROWS // P        # 32 chunks
NPAIR = NCHUNK // 2         # 16 DoubleRow pairs
BGROUPS = [(c, c + 4) for c in range(0, NCHUNK, 4)]  # chunk ranges per DMA
NBN = NP_A * GPP            # 32 bn_stats granules
XMIN, XMAX = 1e-37, 1.0 - 1e-5

_NC_CACHE = {}


def _np_fp8():
    import ml_dtypes
    return ml_dtypes.float8_e4m3fn


def _build_nc():
    """Per-core SPMD program.

    Inputs:  "ht"   [8, 128, 2048] fp8   layout-A pieces (i = b*2 + p)
             "hb"   [32, 128, 512] fp8   layout-B chunks
             "oh8"  [128, 512] fp8       one-hot, col c*16+m <-> row c*128+p
             "oh16" [128, 512] fp16      same in fp16 (for the squared pass)
    Outputs: "outa" [128, 192] f32       bn stats, granule j -> cols j*6..+6
             "outb" [16, 1088] f32       cols 0:512 per-class h sums,
                                         512:1024 per-class h^2 sums (B rows),
                                         1024:1088 warm-up junk
    """
    import concourse.tile as tile
    from concourse import bacc, mybir

    f32 = mybir.dt.float32
    f16 = mybir.dt.float16
    f8 = mybir.dt.float8e4
    AF = mybir.ActivationFunctionType

    nc = bacc.Bacc("TRN2", target_bir_lowering=False, debug=False,
                   num_devices=NCORES)
    ht = nc.declare_dram_parameter("ht", [NP_A, P, PIECE], f8, isOutput=False)
    hb = nc.declare_dram_parameter("hb", [NCHUNK, P, D], f8, isOutput=False)
    oh8 = nc.declare_dram_parameter("oh8", [P, NCHUNK * C], f8, isOutput=False)
    oh16 = nc.declare_dram_parameter("oh16", [P, NCHUNK * C], f16,
                                     isOutput=False)
    outa = nc.declare_dram_parameter("outa", [P, NBN * 6], f32, isOutput=True)
    # last 64 cols hold the PE-warmup dummy result (host ignores them)
    outb = nc.declare_dram_parameter("outb", [C, 2 * D + 64], f32,
                                     isOutput=True)

    with tile.TileContext(nc) as tc:
        with (
            # small pools on purpose: a later piece's DMA descriptor only
            # fires once the buffer's previous tile is consumed, which
            # concentrates HBM bandwidth on the pieces needed next
            tc.tile_pool(name="pa", bufs=5) as pa_pool,
            tc.tile_pool(name="pb", bufs=3) as pb_pool,
            tc.tile_pool(name="sq", bufs=8) as sq_pool,
            tc.tile_pool(name="acc", bufs=1) as acc_pool,
            tc.tile_pool(name="ps", bufs=1, space="PSUM") as psum_pool,
        ):
            bnT = acc_pool.tile([P, NBN * 6], f32, tag="bn")
            evacT = acc_pool.tile([P, 2 * D + 64], f32, tag="evac")
            oh8T = acc_pool.tile([P, NCHUNK * C], f8, tag="oh8")
            oh16T = acc_pool.tile([P, NCHUNK * C], f16, tag="oh16")
            dumT = acc_pool.tile([P, D], f8, tag="dum")
            psum_h = psum_pool.tile([C, D], f32, tag="ph")
            psum_sq = psum_pool.tile([C, D], f32, tag="pq")
            psum_w = psum_pool.tile([C, D], f32, tag="pw")

            nc.vector.memset(dumT[:], 0.0)
            nc.vector.memset(bnT[:], 0.0)

            # ---- DMA kicks on Sync, ordered to match consumption so each
            # engine starts as early as possible (kicks serialize ~0.6us).
            bg = {}

            def kick_b(gi):
                c0, c1 = BGROUPS[gi]
                t = pb_pool.tile([P, c1 - c0, D], f8)
                nc.sync.dma_start(t[:], hb[c0:c1].rearrange("c p f -> p c f"))
                bg[gi] = t

            # DVE is the critical engine: front-load its pieces so its
            # stream never starves (ACT/PE have ~2us more slack)
            s0a = pa_pool.tile([P, G], f8)
            nc.sync.dma_start(
                s0a[:], ht[0].rearrange("p (g x) -> p g x", x=G)[:, 0, :])
            s0b = pa_pool.tile([P, PIECE - G], f8)
            nc.sync.dma_start(s0b[:], ht[0][:, G:])
            a_tiles = {}

            def kick_a(i):
                t = pa_pool.tile([P, PIECE], f8)
                nc.sync.dma_start(t[:], ht[i])
                a_tiles[i] = t

            kick_b(0)
            kick_a(1)
            kick_b(1)
            nc.sync.dma_start(oh8T[:], oh8[:])
            kick_a(2)
            nc.sync.dma_start(oh16T[:], oh16[:])
            kick_b(2)
            kick_a(3)
            kick_b(3)
            kick_a(4)
            kick_b(4)
            kick_a(5)
            kick_b(5)
            kick_a(6)
            kick_b(6)
            kick_a(7)
            kick_b(7)

            # ---- PE warm-up: keep the tensor engine busy through its DMA
            # wait so the clock governor has ramped before the real work
            for w in range(8):
                nc.tensor.matmul(psum_w[:], dumT[:, 0:C], dumT[:],
                                 start=True, stop=True)

            # ---- DVE: bn_stats over all layout-A granules (the ISA caps
            # each instruction at one 512-element group)
            nc.vector.bn_stats(bnT[:, 0:6], s0a[:])
            s0b3 = s0b[:].rearrange("p (g x) -> p g x", x=G)
            for g in range(GPP - 1):
                j = 1 + g
                nc.vector.bn_stats(bnT[:, j * 6:(j + 1) * 6], s0b3[:, g, :])
            for i in range(1, NP_A):
                t3 = a_tiles[i][:].rearrange("p (g x) -> p g x", x=G)
                for g in range(GPP):
                    j = i * GPP + g
                    nc.vector.bn_stats(bnT[:, j * 6:(j + 1) * 6], t3[:, g, :])

            # ---- ACT squares (one big slab per B group: the ~190ns
            # per-instruction overhead amortizes) + PE contractions
            for gi, (c0, c1) in enumerate(BGROUPS):
                ng = c1 - c0
                sq = sq_pool.tile([P, ng, D], f16)
                nc.scalar.activation(sq[:], bg[gi][:], AF.Square)
                for j2 in range(ng // 2):
                    ca, cb = c0 + 2 * j2, c0 + 2 * j2 + 1
                    nc.tensor.matmul(
                        psum_h[:], oh8T[:, ca * C:(cb + 1) * C]
                        .rearrange("p (c m) -> p c m", m=C),
                        bg[gi][:, 2 * j2:2 * j2 + 2, :],
                        start=(ca == 0), stop=(cb == NCHUNK - 1),
                        perf_mode=mybir.MatmulPerfMode.DoubleRow)
                    for c in (ca, cb):
                        nc.tensor.matmul(
                            psum_sq[:], oh16T[:, c * C:(c + 1) * C],
                            sq[:, c - c0, :],
                            start=(c == 0), stop=(c == NCHUNK - 1))

            # ---- evacuate PSUM on ACT (it finishes first; the Vector queue
            # would hold the copy behind all 36 bn_stats), then DMA out
            nc.sync.dma_start(outa[:], bnT[:])
            nc.vector.tensor_copy(evacT[0:C, 2 * D:2 * D + 64],
                                  psum_w[:, 0:64])
            # split the two evacuations across engines so they overlap
            nc.vector.tensor_copy(evacT[0:C, 0:D], psum_h[:])
            nc.scalar.copy(evacT[0:C, D:2 * D], psum_sq[:])
            nc.sync.dma_start(outb[:], evacT[0:C, :])
    nc.compile()
    return nc


def _get_nc():
    if "nc" not in _NC_CACHE:
        _NC_CACHE["nc"] = _build_nc()
    return _NC_CACHE["nc"]


def _prep_core(hs_k, ids_k):
    """hs_k/ids_k globally sorted; returns the device input map."""
    fp8 = _np_fp8()
    T = np.ascontiguousarray(
        hs_k[0:A_ROWS]
        .reshape(NPIECE_A, PIECE, NBLK, P).transpose(2, 0, 3, 1)
        .astype(fp8)
    ).reshape(NP_A, P, PIECE)
    hbm = np.ascontiguousarray(
        hs_k[A_ROWS:].astype(fp8).reshape(NCHUNK, P, D))
    ids_b = np.asarray(ids_k[A_ROWS:]).reshape(NCHUNK, P)
    # oh[p, c*16+m] = 1 iff row c*128+p belongs to class m
    oh = (ids_b[:, :, None] == np.arange(C)[None, None, :])
    oh = np.ascontiguousarray(oh.transpose(1, 0, 2).reshape(P, NCHUNK * C))
    return {
        "ht": T,
        "hb": hbm,
        "oh8": oh.astype(fp8),
        "oh16": oh.astype(np.float16),
    }


def _granules():
    """Yields (bn_col_index, feature_block, row0) for every bn granule."""
    for i in range(NP_A):
        b, p = divmod(i, NPIECE_A)
        for g in range(GPP):
            yield i * GPP + g, b, p * PIECE + g * G


def _core_stats(hs_k, ids_k, dev, sums, sumsq):
    """Accumulate per-class stats for one core into sums/sumsq [C, D] f64."""
    bnr = dev["outa"].astype(np.float64).reshape(P, NBN, 6)
    for j, b, r0 in _granules():
        fsl = slice(b * P, (b + 1) * P)
        r1 = r0 + G
        if ids_k[r0] == ids_k[r1 - 1]:
            cc = int(ids_k[r0])
            ce, me, m2e, co, mo, m2o = bnr[:, j, :].T
            sums[cc, fsl] += ce * me + co * mo
            sumsq[cc, fsl] += m2e + ce * me * me + m2o + co * mo * mo
        else:
            rows = hs_k[r0:r1, fsl].astype(np.float64)
            rids = ids_k[r0:r1]
            for q in np.unique(rids):
                sel = rows[rids == q]
                sums[q, fsl] += sel.sum(axis=0)
                sumsq[q, fsl] += (sel * sel).sum(axis=0)
    outb = dev["outb"].astype(np.float64)
    sums += outb[:, 0:D]
    sumsq += outb[:, D:2 * D]


def _device_stats(hidden, ids, **run_kwargs):
    """Returns (sums[C,D], sumsq[C,D]) float64, plus the raw run result."""
    from concourse import bass_utils

    nc = _get_nc()

    order = np.argsort(ids, kind="stable")       # GLOBAL sort by class
    ids_s = ids[order]
    hs = hidden[order]

    in_maps = []
    for k in range(NCORES):
        rows = slice(k * ROWS, (k + 1) * ROWS)
        in_maps.append(_prep_core(hs[rows], ids_s[rows]))

    res = bass_utils.run_bass_kernel_spmd(nc, in_maps, list(range(NCORES)),
                                          **run_kwargs)

    sums = np.zeros((C, D), dtype=np.float64)
    sumsq = np.zeros((C, D), dtype=np.float64)
    for k in range(NCORES):
        rows = slice(k * ROWS, (k + 1) * ROWS)
        _core_stats(hs[rows], ids_s[rows], res.results[k], sums, sumsq)
    return sums, sumsq, res


def _pairwise_loss(counts, sums, sumsq, d):
    """The tiny O(C^2 D) stage on host CPU.

    Runs in float32 with the same jax ops as the reference: at these extreme
    betainc parameters (b ~ 8190, x ~ 1e-5) jax's f32 betainc differs from
    the true (f64) value by ~1e-3, so matching the reference requires
    replicating its f32 numerics, not improving on them.
    """
    import jax
    import jax.numpy as jnp

    cpu = jax.devices("cpu")[0]
    with jax.default_device(cpu):
        counts64 = counts.astype(np.float64)
        means64 = sums / counts64[:, None]
        withins64 = sumsq - counts64[:, None] * means64**2
        counts = jnp.asarray(counts64, jnp.float32)               # [C]
        means = jnp.asarray(means64, jnp.float32)                 # [C, D]
        withins = jnp.asarray(withins64, jnp.float32)             # [C, D]
        half_diff = (means[:, None, :] - means[None, :, :]) * 0.5
        pair_counts = counts[:, None] + counts[None, :]
        pair_between = half_diff * half_diff * pair_counts[:, :, None]
        pair_within = withins[:, None, :] + withins[None, :, :]
        d2 = pair_counts - 2.0
        d2 = jnp.where(d2 == 0.0, 1e-5, d2)
        x = pair_between / (pair_between + pair_within)
        x = jnp.clip(x, XMIN, XMAX)
        a = jnp.full_like(x, 0.5)
        b = jnp.broadcast_to((d2 * 0.5)[:, :, None], x.shape)
        xbetainc = jax.scipy.special.betainc(a, b, x)             # [C, C, D]
        top_k, _ = jax.lax.top_k(xbetainc, int(d))                # [C, C, d]
        per_pair = jnp.sum(jnp.log(top_k), axis=-1)               # [C, C]
        mask = jnp.triu(jnp.ones((C, C), dtype=bool), k=1)
        total = jnp.sum(jnp.where(mask, per_pair, jnp.zeros_like(per_pair)))
        return float(-total)


def kernel(hidden, batch_ids, d):
    hidden = np.asarray(hidden, dtype=np.float32)
    ids = np.asarray(batch_ids).astype(np.int64)
    assert hidden.shape == (N, D), hidden.shape

    counts = np.bincount(ids, minlength=C).astype(np.float64)
    sums, sumsq, _ = _device_stats(hidden, ids)
    total = _pairwise_loss(counts, sums, sumsq, int(np.asarray(d)))
    return np.array(total, dtype=np.float32)
